# revision 20
# baseline (speedup 1.0000x reference)
"""Trainium2 Bass kernel for causal multi-head attention (dense transformer).

Problem (hardcoded): x [2, 2048, 1024], 16 heads x 64 dh, causal, fp32 I/O.
Sharding: 8 cores = 2 batches x 4 head-groups. Each core computes 4 heads for
one batch plus a partial output projection [2048, 1024] (bf16); the host sums
the 4 partials per batch and adds b_O.

On-device dataflow (all transposed, no transposes anywhere):
  x^T (host-pretransposed)  ->  Q^T, K^T [dh, s] and V [s, dh] via matmuls
  S^T[k, q] = K Q^T  (+ causal -1e5 mask added via identity-matmul accum)
  P^T = exp(S^T / 8) on ACT straight into SBUF bf16 (no separate masking)
  Zu^T[dh, q] and the softmax denominator come from ONE matmul per k-tile:
    V is stored with a ones column appended ([V_A | 1] / [1 | V_B], 65 wide),
    so the PV matmul emits 64 rows of Z plus 1 row of sum(P).
  denominators: copied out, reciprocal'd, broadcast across partitions with a
    tiny ones-matmul, then Z is normalized while copying to SBUF.
  O[s, :] = (Z^T)^T W_O  (Z^T is directly the lhsT of the O-projection)

Heads are processed in pairs: QK^T packs 2 heads in row-groups (0-63/64-127)
of the PE array (concurrent); PV runs per head with 65-wide stationary weights.
"""

import os
from contextlib import ExitStack

import numpy as np

import concourse.tile as tile
from concourse import bacc, mybir
from concourse.bass_utils import run_bass_kernel_spmd

# problem constants
B, S, DM, H, DH = 2, 2048, 1024, 16, 64
P = 128          # partitions
QB = 512         # q block (matmul moving free dim)
NKT = S // P     # 16 k tiles
NQB = S // QB    # 4 q blocks
NDM = DM // P    # 8 d_model tiles
HPC = 4          # heads per core
NCORES = 8
MASKV = -99840.0  # additive causal mask (bf16-exact); exp(MASKV/8) == 0
VW = 2 * (DH + 1)  # 130: per-pair padded V width [V_A|0 , 0|V_B]

F32 = mybir.dt.float32
BF16 = mybir.dt.bfloat16

_PROGRAM_CACHE = {}
LAST_RESULTS = None  # BassKernelResults of the most recent run (for test.py)


def _mm(nc, out, lhsT, rhs, start, stop, skip=False):
    return nc.tensor.matmul(
        out, lhsT, rhs, start=start, stop=stop, skip_group_check=skip
    )


def _chain(insts):
    """Ordering-only PE edges so row-group-packed matmuls stay adjacent and
    run concurrently on the array."""
    from concourse.tile import add_dep_helper

    for a, b in zip(insts[1:], insts):
        add_dep_helper(a.ins, b.ins, sync=False, reason="pack-pair order")


def build_program():
    if "p" in _PROGRAM_CACHE:
        return _PROGRAM_CACHE["p"]

    nc = bacc.Bacc(
        "TRN2", target_bir_lowering=False, debug=False, num_devices=NCORES
    )

    # ---- DRAM I/O (per-core shards, prearranged on host) ----
    xT_d = nc.dram_tensor("xT", [DM, S], BF16, kind="ExternalInput")
    wq_d = nc.dram_tensor("wq", [DM, HPC * DH], BF16, kind="ExternalInput")
    wk_d = nc.dram_tensor("wk", [DM, HPC * DH], BF16, kind="ExternalInput")
    wv_d = nc.dram_tensor("wv", [DM, 2 * VW], BF16, kind="ExternalInput")
    wo_d = nc.dram_tensor("wo", [HPC * DH, DM], BF16, kind="ExternalInput")
    bq_d = nc.dram_tensor("bq", [2, P], F32, kind="ExternalInput")
    bk_d = nc.dram_tensor("bk", [2, P], F32, kind="ExternalInput")
    bv_d = nc.dram_tensor("bv", [P, 2 * VW], F32, kind="ExternalInput")
    maskm_d = nc.dram_tensor("maskm", [2, P, 2 * QB], BF16, kind="ExternalInput")
    id_d = nc.dram_tensor("idm", [P, P], BF16, kind="ExternalInput")
    out_d = nc.dram_tensor("out", [S, DM], BF16, kind="ExternalOutput")

    with tile.TileContext(nc) as tc, ExitStack() as ctx:
        const = ctx.enter_context(tc.tile_pool(name="const", bufs=1))
        persist = ctx.enter_context(tc.tile_pool(name="persist", bufs=1))

        # ---- constants ----
        # row of ones at partition 64 (matmul lhsT/rhs must share base)
        ones1 = const.tile([DH + 1, DH], BF16, name="ones1", tag="ones1")
        nc.gpsimd.memset(ones1[DH : DH + 1, :], 1.0)
        maskm_sb = const.tile([P, 2, 2 * QB], BF16, name="maskm_sb", tag="maskm")
        id_sb = const.tile([P, P], BF16, name="id_sb", tag="idm")
        bq_sb = const.tile([P, 2], F32, name="bq_sb", tag="bq")
        bk_sb = const.tile([P, 2], F32, name="bk_sb", tag="bk")
        bv_sb = const.tile([P, 2 * VW], F32, name="bv_sb", tag="bv")

        # ---- persistent activations ----
        qt_sb = [persist.tile([P, S], BF16, name=f"qt{p}", tag=f"qt{p}")
                 for p in range(2)]
        kt_sb = [persist.tile([P, S], BF16, name=f"kt{p}", tag=f"kt{p}")
                 for p in range(2)]
        # V with ones column: [:, kt, 0, 0:64]=V_A, [:, kt, 0, 64]=1,
        #                     [:, kt, 1, 0]=1,     [:, kt, 1, 1:65]=V_B
        v65_sb = [persist.tile([P, NKT, 2, DH + 1], BF16, name=f"v{p}",
                               tag=f"v{p}") for p in range(2)]
        zt_sb = [persist.tile([P, S], BF16, name=f"zt{p}", tag=f"zt{p}")
                 for p in range(2)]
        wo_sb = persist.tile([P, 2, DM], BF16, name="wo_sb", tag="wo")

        # ---- psum pools: sp (scores/proj/den_bc/O) 2x2 banks, zup 2x2 ----
        sp = ctx.enter_context(tc.tile_pool(name="sp", bufs=2, space="PSUM"))
        zup = ctx.enter_context(tc.tile_pool(name="zup", bufs=2, space="PSUM"))
        xw = ctx.enter_context(tc.tile_pool(name="xw", bufs=1))
        ppool = ctx.enter_context(tc.tile_pool(name="ppool", bufs=6))
        denp = ctx.enter_context(tc.tile_pool(name="denp", bufs=2))
        bcpool = ctx.enter_context(tc.tile_pool(name="bcpool", bufs=2))
        ost = ctx.enter_context(tc.tile_pool(name="ost", bufs=3))

        # ---- input DMAs: weights first, x column-major so the first
        # projection chunk can start after ~1.5MB instead of 4MB ----
        xt_sb = xw.tile([P, NDM, S], BF16, name="xt_sb", tag="xt")
        w_sb = {
            wname: xw.tile([P, NDM, w], BF16, name=f"{wname}_sb", tag=wname)
            for wname, w in (("wq", HPC * DH), ("wk", HPC * DH), ("wv", 2 * VW))
        }
        for t in range(NDM):
            nc.sync.dma_start(out=w_sb["wq"][:, t, :], in_=wq_d[t * P : (t + 1) * P, :])
        for t in range(NDM):
            nc.sync.dma_start(out=xt_sb[:, t, 0:QB], in_=xT_d[t * P : (t + 1) * P, 0:QB])
        for t in range(NDM):
            nc.sync.dma_start(out=w_sb["wk"][:, t, :], in_=wk_d[t * P : (t + 1) * P, :])
        for t in range(NDM):
            nc.sync.dma_start(out=xt_sb[:, t, QB : 2 * QB],
                              in_=xT_d[t * P : (t + 1) * P, QB : 2 * QB])
        for t in range(NDM):
            nc.sync.dma_start(out=w_sb["wv"][:, t, :], in_=wv_d[t * P : (t + 1) * P, :])
        for ch in range(2, NQB):
            for t in range(NDM):
                nc.sync.dma_start(out=xt_sb[:, t, ch * QB : (ch + 1) * QB],
                                  in_=xT_d[t * P : (t + 1) * P, ch * QB : (ch + 1) * QB])
        # small consts + wo after the bulk
        nc.sync.dma_start(out=id_sb[:], in_=id_d[:, :])
        for o in range(2):
            nc.sync.dma_start(out=maskm_sb[:, o, :], in_=maskm_d[o, :, :])
        for p in range(2):
            nc.sync.dma_start(out=bq_sb[:, p : p + 1], in_=bq_d[p : p + 1, :])
            nc.sync.dma_start(out=bk_sb[:, p : p + 1], in_=bk_d[p : p + 1, :])
        nc.sync.dma_start(out=bv_sb[:], in_=bv_d[:, :])
        for p in range(2):
            nc.sync.dma_start(out=wo_sb[:, p, :], in_=wo_d[p * P : (p + 1) * P, :])

        def qk_proj(p):
            # Q^T and K^T for pair p: [dh-pair (128), seq]; rows 0-63 =
            # head 2p, 64-127 = head 2p+1
            for dst, wname, bias in ((qt_sb, "wq", bq_sb), (kt_sb, "wk", bk_sb)):
                for ch in range(NQB):
                    qp = sp.tile([P, 2, QB], F32, name="qp", tag="s")
                    for t in range(NDM):
                        _mm(nc, qp[:, 0, :],
                            w_sb[wname][:, t, p * P : (p + 1) * P],
                            xt_sb[:, t, ch * QB : (ch + 1) * QB],
                            start=(t == 0), stop=(t == NDM - 1))
                    nc.vector.tensor_scalar_add(
                        dst[p][:, ch * QB : (ch + 1) * QB], qp[:, 0, :],
                        bias[:, p : p + 1])

        def v_proj():
            # V: [seq, per-pair [V_A|0 , 0|V_B]] per 128-row tile; the ones
            # columns come from the padded bias (wv cols are 0 there)
            for st in range(NKT):
                vp = sp.tile([P, 2, QB], F32, name="vp", tag="s")
                for t in range(NDM):
                    _mm(nc, vp[:, 0, 0 : 2 * VW],
                        xt_sb[:, t, st * P : (st + 1) * P],
                        w_sb["wv"][:, t, :],
                        start=(t == 0), stop=(t == NDM - 1))
                for p in range(2):
                    nc.vector.tensor_add(
                        v65_sb[p][:, st, :, :],
                        vp[:, 0, p * VW : (p + 1) * VW].rearrange(
                            "p (a b) -> p a b", b=DH + 1),
                        bv_sb[:, p * VW : (p + 1) * VW].rearrange(
                            "p (a b) -> p a b", b=DH + 1))

        def attn_qblock(p, qb, zu):
            """scores+mask -> exp -> PV for pair p, q-block qb.
            Emits with 1-kg software pipelining (PV lags one k-group)."""
            q0 = qb * QB
            nk = (qb + 1) * (QB // P)     # k tiles in causal range
            prev = None                    # (pA, pB, kg, c0) pending PV

            def pv(pA, pB, kg):
                for j in range(2):
                    kt = kg * 2 + j
                    c0 = max(kt * P - q0, 0)
                    _mm(nc, zu[0 : DH + 1, 0, c0:QB],
                        v65_sb[p][:, kt, 0, :], pA[:, j, c0:QB],
                        start=(kt == 0), stop=(kt == nk - 1))
                    _mm(nc, zu[0 : DH + 1, 1, c0:QB],
                        v65_sb[p][:, kt, 1, :], pB[:, j, c0:QB],
                        start=(kt == 0), stop=(kt == nk - 1))

            for kg in range(nk // 2):
                off0 = kg * 2 * P - q0    # first valid col of k-tile j=0
                band = off0 >= 0
                c0 = max(off0, 0)
                o = off0 // (2 * P) if band else 0
                sA = sp.tile([P, 2, QB], F32, name="sA", tag="s")
                sB = sp.tile([P, 2, QB], F32, name="sB", tag="s")
                for j in range(2):
                    _chain([
                        _mm(nc, stile[:, j, c0:QB],
                            kt_sb[p][rows, (kg * 2 + j) * P : (kg * 2 + j + 1) * P],
                            qt_sb[p][rows, q0 + c0 : q0 + QB],
                            start=True, stop=not band)
                        for rows, stile in ((slice(0, 64), sA), (slice(64, P), sB))
                    ])
                if band:
                    # additive causal mask via identity-matmul accumulation
                    for j in range(2):
                        for stile in (sA, sB):
                            _mm(nc, stile[:, j, c0:QB], id_sb[:],
                                maskm_sb[:, o, j * QB + c0 : (j + 1) * QB],
                                start=False, stop=True)
                pA = ppool.tile([P, 2, QB], BF16, name="pA", tag="pt")
                pB = ppool.tile([P, 2, QB], BF16, name="pB", tag="pt")
                nc.scalar.activation(pA[:, :, c0:QB], sA[:, :, c0:QB],
                                     mybir.ActivationFunctionType.Exp,
                                     scale=0.125)
                nc.scalar.activation(pB[:, :, c0:QB], sB[:, :, c0:QB],
                                     mybir.ActivationFunctionType.Exp,
                                     scale=0.125)
                if prev is not None:
                    pv(*prev)
                prev = (pA, pB, kg)
            pv(*prev)

        def den_chain(p, qb, zu):
            """denominator rows -> bf16 -> reciprocal of PE-broadcast -> Z."""
            q0 = qb * QB
            den16 = denp.tile([DH + 1, 2, QB], BF16, name="den16", tag="den")
            # zu[64,0]=den_A, zu[64,1]=den_B; copy at matching partition 64
            nc.vector.tensor_copy(den16[DH : DH + 1, :, :],
                                  zu[DH : DH + 1, :, :])
            dbc = sp.tile([P, 2, QB], F32, name="dbc", tag="s")
            _mm(nc, dbc[0:DH, 0, :], ones1[DH : DH + 1, :],
                den16[DH : DH + 1, 0, :], start=True, stop=True, skip=True)
            _mm(nc, dbc[DH:P, 0, :], ones1[DH : DH + 1, :],
                den16[DH : DH + 1, 1, :], start=True, stop=True, skip=True)
            bcr = bcpool.tile([P, QB], F32, name="bcr", tag="bcr")
            nc.vector.reciprocal_approx_fast(out=bcr[:], in_=dbc[:, 0, :])
            nc.vector.tensor_mul(zt_sb[p][0:DH, q0 : q0 + QB],
                                 zu[0:DH, 0, :], bcr[0:DH, :])
            nc.vector.tensor_mul(zt_sb[p][DH:P, q0 : q0 + QB],
                                 zu[0:DH, 1, :], bcr[DH:P, :])

        def o_proj(qb):
            for st in range(qb * (QB // P), (qb + 1) * (QB // P)):
                op = sp.tile([P, 2, QB], F32, name="op", tag="s")
                for nn in range(2):
                    for pp in range(2):
                        _mm(nc, op[:, nn, :],
                            zt_sb[pp][:, st * P : (st + 1) * P],
                            wo_sb[:, pp, nn * QB : (nn + 1) * QB],
                            start=(pp == 0), stop=(pp == 1))
                ot = ost.tile([P, 2, QB], BF16, name="ot", tag="ot")
                nc.vector.tensor_copy(ot[:], op[:])
                nc.sync.dma_start(
                    out=out_d[st * P : (st + 1) * P, :],
                    in_=ot[:].rearrange("p a b -> p (a b)"))

        qk_proj(0)
        v_proj()
        qk_proj(1)

        for qb in range(NQB):
            zu0 = zup.tile([P, 2, QB], F32, name="zu", tag="z")
            attn_qblock(0, qb, zu0)
            zu1 = zup.tile([P, 2, QB], F32, name="zu", tag="z")
            attn_qblock(1, qb, zu1)
            den_chain(0, qb, zu0)
            if qb > 0:
                o_proj(qb - 1)
            den_chain(1, qb, zu1)
        o_proj(NQB - 1)

    nc.compile()
    _PROGRAM_CACHE["p"] = nc
    return nc


def make_in_maps(normalized_resid_pre, W_Q, W_K, W_V, W_O, b_Q, b_K, b_V, b_O):
    """Shard + prearrange the full inputs into per-core input maps."""
    import ml_dtypes  # noqa: F401  (registers bfloat16 with numpy)

    np_bf = np.dtype("bfloat16")
    x = np.asarray(normalized_resid_pre, dtype=np.float32)
    W_Q = np.asarray(W_Q, dtype=np.float32)
    W_K = np.asarray(W_K, dtype=np.float32)
    W_V = np.asarray(W_V, dtype=np.float32)
    W_O = np.asarray(W_O, dtype=np.float32)
    b_Q = np.asarray(b_Q, dtype=np.float32)
    b_K = np.asarray(b_K, dtype=np.float32)
    b_V = np.asarray(b_V, dtype=np.float32)

    xT = [np.ascontiguousarray(x[b].T).astype(np_bf) for b in range(B)]
    # additive causal masks at k-group granularity: variant o covers the two
    # k-tiles at q-block offsets (2o*128, (2o+1)*128)
    kp = np.arange(P)[:, None]
    qc = np.arange(QB)[None, :]
    maskm = np.stack([
        np.concatenate([
            np.where(qc < (2 * o + j) * P + kp, np.float32(MASKV),
                     np.float32(0.0))
            for j in range(2)
        ], axis=1)
        for o in range(2)
    ]).astype(np_bf)
    idm = np.eye(P, dtype=np.float32).astype(np_bf)

    in_maps = []
    for c in range(NCORES):
        b = c // (NCORES // B)
        heads = [HPC * (c % (NCORES // B)) + i for i in range(HPC)]
        wq = np.concatenate([W_Q[h] for h in heads], axis=1).astype(np_bf)
        wk = np.concatenate([W_K[h] for h in heads], axis=1).astype(np_bf)
        # per head slot: [V_h | 0col]; the ones column comes from the bias
        zc = np.zeros((DM, 1), dtype=np.float32)
        wv = np.concatenate(
            sum(([W_V[h], zc] for h in heads), []), axis=1).astype(np_bf)
        wo = np.concatenate([W_O[h] for h in heads], axis=0).astype(np_bf)
        bq = np.stack([
            np.concatenate([b_Q[heads[0]], b_Q[heads[1]]]),
            np.concatenate([b_Q[heads[2]], b_Q[heads[3]]]),
        ]).astype(np.float32)
        bk = np.stack([
            np.concatenate([b_K[heads[0]], b_K[heads[1]]]),
            np.concatenate([b_K[heads[2]], b_K[heads[3]]]),
        ]).astype(np.float32)
        one = np.ones((1,), dtype=np.float32)
        bv = np.tile(np.concatenate(
            sum(([b_V[h], one] for h in heads), []))[None, :],
            (P, 1)).astype(np.float32)
        in_maps.append({
            "xT": np.ascontiguousarray(xT[b]),
            "wq": wq, "wk": wk, "wv": wv, "wo": wo,
            "bq": bq, "bk": bk, "bv": bv,
            "maskm": maskm, "idm": idm,
        })
    return in_maps


def kernel(normalized_resid_pre, W_Q, W_K, W_V, W_O, b_Q, b_K, b_V, b_O):
    global LAST_RESULTS
    nc = build_program()
    in_maps = make_in_maps(
        normalized_resid_pre, W_Q, W_K, W_V, W_O, b_Q, b_K, b_V, b_O
    )
    trace = os.environ.get("ATTN_TRACE", "0") == "1"
    res = run_bass_kernel_spmd(nc, in_maps, list(range(NCORES)), trace=trace)
    LAST_RESULTS = res

    b_O = np.asarray(b_O, dtype=np.float32)
    parts = [np.asarray(res.results[c]["out"], dtype=np.float64)
             for c in range(NCORES)]
    npc = NCORES // B  # cores per batch
    out = np.stack(
        [sum(parts[b * npc : (b + 1) * npc]) + b_O for b in range(B)]
    )
    return out.astype(np.float32)


# revision 24
# speedup vs baseline: 1.0543x; 1.0543x over previous
"""Trainium2 Bass kernel for causal multi-head attention (dense transformer).

Problem (hardcoded): x [2, 2048, 1024], 16 heads x 64 dh, causal, fp32 I/O.
Sharding: 8 cores = 2 batches x 4 head-groups. Each core computes 4 heads for
one batch plus a partial output projection [2048, 1024] (bf16); the host sums
the 4 partials per batch and adds b_O.

On-device dataflow (all transposed, no transposes anywhere):
  x^T (host-pretransposed)  ->  Q^T, K^T [dh, s] and V [s, dh] via matmuls
  S^T[k, q] = K Q^T  (+ causal -1e5 mask added via identity-matmul accum)
  P^T = exp(S^T / 8) on ACT straight into SBUF bf16 (no separate masking)
  Zu^T[dh, q] and the softmax denominator come from ONE matmul per k-tile:
    V is stored with a ones column appended ([V_A | 1] / [1 | V_B], 65 wide),
    so the PV matmul emits 64 rows of Z plus 1 row of sum(P).
  denominators: copied out, reciprocal'd, broadcast across partitions with a
    tiny ones-matmul, then Z is normalized while copying to SBUF.
  O[s, :] = (Z^T)^T W_O  (Z^T is directly the lhsT of the O-projection)

Heads are processed in pairs: QK^T packs 2 heads in row-groups (0-63/64-127)
of the PE array (concurrent); PV runs per head with 65-wide stationary weights.
"""

import os
from contextlib import ExitStack

import numpy as np

import concourse.tile as tile
from concourse import bacc, mybir
from concourse.bass_utils import run_bass_kernel_spmd

# problem constants
B, S, DM, H, DH = 2, 2048, 1024, 16, 64
P = 128          # partitions
QB = 512         # q block (matmul moving free dim)
NKT = S // P     # 16 k tiles
NQB = S // QB    # 4 q blocks
NDM = DM // P    # 8 d_model tiles
HPC = 4          # heads per core
NCORES = 8
MASKV = -99840.0  # additive causal mask (bf16-exact); exp(MASKV/8) == 0
VW = 2 * (DH + 1)  # 130: per-pair padded V width [V_A|0 , 0|V_B]

F32 = mybir.dt.float32
BF16 = mybir.dt.bfloat16

_PROGRAM_CACHE = {}
LAST_RESULTS = None  # BassKernelResults of the most recent run (for test.py)


def _mm(nc, out, lhsT, rhs, start, stop, skip=False):
    return nc.tensor.matmul(
        out, lhsT, rhs, start=start, stop=stop, skip_group_check=skip
    )


def _chain(insts):
    """Ordering-only PE edges so row-group-packed matmuls stay adjacent and
    run concurrently on the array."""
    from concourse.tile import add_dep_helper

    for a, b in zip(insts[1:], insts):
        add_dep_helper(a.ins, b.ins, sync=False, reason="pack-pair order")


def build_program():
    if "p" in _PROGRAM_CACHE:
        return _PROGRAM_CACHE["p"]

    nc = bacc.Bacc(
        "TRN2", target_bir_lowering=False, debug=False, num_devices=NCORES
    )

    # ---- DRAM I/O (per-core shards, prearranged on host) ----
    xT_d = nc.dram_tensor("xT", [DM, S], BF16, kind="ExternalInput")
    wq_d = nc.dram_tensor("wq", [DM, HPC * DH], BF16, kind="ExternalInput")
    wk_d = nc.dram_tensor("wk", [DM, HPC * DH], BF16, kind="ExternalInput")
    wv_d = nc.dram_tensor("wv", [DM, 2 * VW], BF16, kind="ExternalInput")
    wo_d = nc.dram_tensor("wo", [HPC * DH, DM], BF16, kind="ExternalInput")
    bq_d = nc.dram_tensor("bq", [2, P], F32, kind="ExternalInput")
    bk_d = nc.dram_tensor("bk", [2, P], F32, kind="ExternalInput")
    bv_d = nc.dram_tensor("bv", [P, 2 * VW], F32, kind="ExternalInput")
    maskm_d = nc.dram_tensor("maskm", [2, P, 2 * QB], BF16, kind="ExternalInput")
    id_d = nc.dram_tensor("idm", [P, P], BF16, kind="ExternalInput")
    out_d = nc.dram_tensor("out", [S, DM], BF16, kind="ExternalOutput")

    with tile.TileContext(nc) as tc, ExitStack() as ctx:
        const = ctx.enter_context(tc.tile_pool(name="const", bufs=1))
        persist = ctx.enter_context(tc.tile_pool(name="persist", bufs=1))

        # ---- constants ----
        # row of ones at partition 64 (matmul lhsT/rhs must share base)
        ones1 = const.tile([DH + 1, DH], BF16, name="ones1", tag="ones1")
        nc.gpsimd.memset(ones1[DH : DH + 1, :], 1.0)
        maskm_sb = const.tile([P, 2, 2 * QB], BF16, name="maskm_sb", tag="maskm")
        id_sb = const.tile([P, P], BF16, name="id_sb", tag="idm")
        bq_sb = const.tile([P, 2], F32, name="bq_sb", tag="bq")
        bk_sb = const.tile([P, 2], F32, name="bk_sb", tag="bk")
        bv_sb = const.tile([P, 2 * VW], F32, name="bv_sb", tag="bv")

        # ---- persistent activations ----
        qt_sb = [persist.tile([P, S], BF16, name=f"qt{p}", tag=f"qt{p}")
                 for p in range(2)]
        kt_sb = [persist.tile([P, S], BF16, name=f"kt{p}", tag=f"kt{p}")
                 for p in range(2)]
        # V with ones column: [:, kt, 0, 0:64]=V_A, [:, kt, 0, 64]=1,
        #                     [:, kt, 1, 0]=1,     [:, kt, 1, 1:65]=V_B
        v65_sb = [persist.tile([P, NKT, 2, DH + 1], BF16, name=f"v{p}",
                               tag=f"v{p}") for p in range(2)]
        zt_sb = [persist.tile([P, S], BF16, name=f"zt{p}", tag=f"zt{p}")
                 for p in range(2)]
        wo_sb = persist.tile([P, 2, DM], BF16, name="wo_sb", tag="wo")

        # ---- psum pools: sp (scores/proj/den_bc/O) 2x2 banks, zup 2x2 ----
        sp = ctx.enter_context(tc.tile_pool(name="sp", bufs=2, space="PSUM"))
        zup = ctx.enter_context(tc.tile_pool(name="zup", bufs=2, space="PSUM"))
        xw = ctx.enter_context(tc.tile_pool(name="xw", bufs=1))
        ppool = ctx.enter_context(tc.tile_pool(name="ppool", bufs=6))
        denp = ctx.enter_context(tc.tile_pool(name="denp", bufs=2))
        bcpool = ctx.enter_context(tc.tile_pool(name="bcpool", bufs=2))
        ost = ctx.enter_context(tc.tile_pool(name="ost", bufs=3))

        # ---- input DMAs: one big strided DMA per tensor / x column-chunk
        # (each dma_start costs ~0.65us of serial sync-engine issue time) ----
        xt_sb = xw.tile([P, NDM, S], BF16, name="xt_sb", tag="xt")
        w_sb = {
            wname: xw.tile([P, NDM, w], BF16, name=f"{wname}_sb", tag=wname)
            for wname, w in (("wq", HPC * DH), ("wk", HPC * DH), ("wv", 2 * VW))
        }

        def dma_w(wname, src):
            nc.sync.dma_start(
                out=w_sb[wname][:],
                in_=src[:, :].rearrange("(t p) c -> p t c", p=P))

        def dma_x(ch):
            nc.sync.dma_start(
                out=xt_sb[:, :, ch * QB : (ch + 1) * QB],
                in_=xT_d[:, ch * QB : (ch + 1) * QB].rearrange(
                    "(t p) s -> p t s", p=P))

        dma_w("wq", wq_d)
        dma_x(0)
        dma_w("wk", wk_d)
        nc.sync.dma_start(out=id_sb[:], in_=id_d[:, :])
        nc.sync.dma_start(out=maskm_sb[:],
                          in_=maskm_d[:, :, :].rearrange("o p c -> p o c"))
        nc.sync.dma_start(out=bq_sb[:], in_=bq_d[:, :].rearrange("a p -> p a"))
        nc.sync.dma_start(out=bk_sb[:], in_=bk_d[:, :].rearrange("a p -> p a"))
        nc.sync.dma_start(out=bv_sb[:], in_=bv_d[:, :])
        dma_w("wv", wv_d)
        for ch in range(1, NQB):
            dma_x(ch)
        nc.sync.dma_start(out=wo_sb[:],
                          in_=wo_d[:, :].rearrange("(pp p) d -> p pp d", p=P))

        def qk_unit(p, wname, dst, bias, ch):
            # Q^T or K^T chunk for pair p: [dh-pair (128), 512 seq cols];
            # rows 0-63 = head 2p, 64-127 = head 2p+1
            def emit():
                qp = sp.tile([P, 2, QB], F32, name="qp", tag="s")
                for t in range(NDM):
                    _mm(nc, qp[:, 0, :],
                        w_sb[wname][:, t, p * P : (p + 1) * P],
                        xt_sb[:, t, ch * QB : (ch + 1) * QB],
                        start=(t == 0), stop=(t == NDM - 1))
                nc.vector.tensor_scalar_add(
                    dst[p][:, ch * QB : (ch + 1) * QB], qp[:, 0, :],
                    bias[:, p : p + 1])
            return emit

        def v_unit(st):
            # V seq-tile st: [seq 128, per-head [V_h|0]]; the ones columns
            # come from the padded bias (wv cols are 0 there)
            def emit():
                vp = sp.tile([P, 2, QB], F32, name="vp", tag="s")
                for t in range(NDM):
                    _mm(nc, vp[:, 0, 0 : 2 * VW],
                        xt_sb[:, t, st * P : (st + 1) * P],
                        w_sb["wv"][:, t, :],
                        start=(t == 0), stop=(t == NDM - 1))
                for p in range(2):
                    nc.vector.tensor_add(
                        v65_sb[p][:, st, :, :],
                        vp[:, 0, p * VW : (p + 1) * VW].rearrange(
                            "p (a b) -> p a b", b=DH + 1),
                        bv_sb[:, p * VW : (p + 1) * VW].rearrange(
                            "p (a b) -> p a b", b=DH + 1))
            return emit

        def o_unit(st):
            # output-projection seq-tile st (both 512-col halves)
            def emit():
                op = sp.tile([P, 2, QB], F32, name="op", tag="s")
                for nn in range(2):
                    for pp in range(2):
                        _mm(nc, op[:, nn, :],
                            zt_sb[pp][:, st * P : (st + 1) * P],
                            wo_sb[:, pp, nn * QB : (nn + 1) * QB],
                            start=(pp == 0), stop=(pp == 1))
                ot = ost.tile([P, 2, QB], BF16, name="ot", tag="ot")
                nc.vector.tensor_copy(ot[:], op[:])
                nc.gpsimd.dma_start(
                    out=out_d[st * P : (st + 1) * P, :],
                    in_=ot[:].rearrange("p a b -> p (a b)"))
            return emit

        def attn_qblock(p, qb, zu, fill, pre=None):
            """scores+mask -> exp -> PV for pair p, q-block qb, with 1-kg
            software pipelining (PV lags one k-group). `fill()` is called
            once per k-group to emit interleaved PE filler work; `pre` is
            emitted between the first sc/exp and the first PV (den chain of
            the other pair)."""
            q0 = qb * QB
            nk = (qb + 1) * (QB // P)     # k tiles in causal range
            prev = None                    # (pA, pB, kg) pending PV

            def pv(pA, pB, kg):
                for j in range(2):
                    kt = kg * 2 + j
                    c0 = max(kt * P - q0, 0)
                    _mm(nc, zu[0 : DH + 1, 0, c0:QB],
                        v65_sb[p][:, kt, 0, :], pA[:, j, c0:QB],
                        start=(kt == 0), stop=(kt == nk - 1))
                    _mm(nc, zu[0 : DH + 1, 1, c0:QB],
                        v65_sb[p][:, kt, 1, :], pB[:, j, c0:QB],
                        start=(kt == 0), stop=(kt == nk - 1))

            for kg in range(nk // 2):
                off0 = kg * 2 * P - q0    # first valid col of k-tile j=0
                band = off0 >= 0
                c0 = max(off0, 0)
                o = off0 // (2 * P) if band else 0
                sA = sp.tile([P, 2, QB], F32, name="sA", tag="s")
                sB = sp.tile([P, 2, QB], F32, name="sB", tag="s")
                for j in range(2):
                    _chain([
                        _mm(nc, stile[:, j, c0:QB],
                            kt_sb[p][rows, (kg * 2 + j) * P : (kg * 2 + j + 1) * P],
                            qt_sb[p][rows, q0 + c0 : q0 + QB],
                            start=True, stop=not band)
                        for rows, stile in ((slice(0, 64), sA), (slice(64, P), sB))
                    ])
                if band:
                    # additive causal mask via identity-matmul accumulation
                    for j in range(2):
                        for stile in (sA, sB):
                            _mm(nc, stile[:, j, c0:QB], id_sb[:],
                                maskm_sb[:, o, j * QB + c0 : (j + 1) * QB],
                                start=False, stop=True)
                pA = ppool.tile([P, 2, QB], BF16, name="pA", tag="pt")
                pB = ppool.tile([P, 2, QB], BF16, name="pB", tag="pt")
                nc.scalar.activation(pA[:, :, c0:QB], sA[:, :, c0:QB],
                                     mybir.ActivationFunctionType.Exp,
                                     scale=0.125)
                nc.scalar.activation(pB[:, :, c0:QB], sB[:, :, c0:QB],
                                     mybir.ActivationFunctionType.Exp,
                                     scale=0.125)
                if pre is not None:
                    pre()
                    pre = None
                fill()
                if prev is not None:
                    pv(*prev)
                prev = (pA, pB, kg)
            pv(*prev)

        def den_chain(p, qb, zu):
            """denominator rows -> bf16 -> reciprocal of PE-broadcast -> Z."""
            q0 = qb * QB
            den16 = denp.tile([DH + 1, 2, QB], BF16, name="den16", tag="den")
            # zu[64,0]=den_A, zu[64,1]=den_B; copy at matching partition 64
            nc.vector.tensor_copy(den16[DH : DH + 1, :, :],
                                  zu[DH : DH + 1, :, :])
            dbc = sp.tile([P, 2, QB], F32, name="dbc", tag="s")
            _mm(nc, dbc[0:DH, 0, :], ones1[DH : DH + 1, :],
                den16[DH : DH + 1, 0, :], start=True, stop=True, skip=True)
            _mm(nc, dbc[DH:P, 0, :], ones1[DH : DH + 1, :],
                den16[DH : DH + 1, 1, :], start=True, stop=True, skip=True)
            bcr = bcpool.tile([P, QB], F32, name="bcr", tag="bcr")
            nc.vector.reciprocal_approx_fast(out=bcr[:], in_=dbc[:, 0, :])
            nc.vector.tensor_mul(zt_sb[p][0:DH, q0 : q0 + QB],
                                 zu[0:DH, 0, :], bcr[0:DH, :])
            nc.vector.tensor_mul(zt_sb[p][DH:P, q0 : q0 + QB],
                                 zu[0:DH, 1, :], bcr[DH:P, :])

        class Filler:
            """Evenly drains a unit list across an attn q-block's k-groups."""
            def __init__(self, units, slots):
                self.units = list(units)
                self.slots = slots

            def __call__(self):
                n = -(-len(self.units) // self.slots) if self.slots > 0 else 0
                for u in self.units[:n]:
                    u()
                del self.units[:n]
                self.slots -= 1

        def qkp(p, ch):
            return [qk_unit(p, "wq", qt_sb, bq_sb, ch),
                    qk_unit(p, "wk", kt_sb, bk_sb, ch)]

        # prologue: just enough projection for attn(0, qb0) to start
        for u in qkp(0, 0) + [v_unit(0), v_unit(1)]:
            u()

        # fills for [attn(0, qb), attn(1, qb)] segments; O(st) units appear
        # one q-block after their zt is finalized
        fills = {
            (0, 0): [v_unit(2), v_unit(3)] + qkp(1, 0),
            (1, 0): qkp(0, 1),
            (0, 1): qkp(1, 1) + [v_unit(s) for s in range(4, 8)],
            (1, 1): qkp(0, 2) + [o_unit(0), o_unit(1)],
            (0, 2): qkp(1, 2) + [v_unit(s) for s in range(8, 12)]
                    + [o_unit(2), o_unit(3)],
            (1, 2): qkp(0, 3) + [o_unit(4), o_unit(5)],
            (0, 3): qkp(1, 3) + [v_unit(s) for s in range(12, 16)]
                    + [o_unit(6), o_unit(7)],
            (1, 3): [o_unit(s) for s in range(8, 12)],
        }

        for qb in range(NQB):
            nslots = (qb + 1) * 2
            zu0 = zup.tile([P, 2, QB], F32, name="zu", tag="z")
            attn_qblock(0, qb, zu0, Filler(fills[(0, qb)], nslots))
            zu1 = zup.tile([P, 2, QB], F32, name="zu", tag="z")
            attn_qblock(1, qb, zu1, Filler(fills[(1, qb)], nslots),
                        pre=lambda q=qb, z=zu0: den_chain(0, q, z))
            den_chain(1, qb, zu1)
        for st in range(12, 16):
            o_unit(st)()

    nc.compile()
    _PROGRAM_CACHE["p"] = nc
    return nc


def make_in_maps(normalized_resid_pre, W_Q, W_K, W_V, W_O, b_Q, b_K, b_V, b_O):
    """Shard + prearrange the full inputs into per-core input maps."""
    import ml_dtypes  # noqa: F401  (registers bfloat16 with numpy)

    np_bf = np.dtype("bfloat16")
    x = np.asarray(normalized_resid_pre, dtype=np.float32)
    W_Q = np.asarray(W_Q, dtype=np.float32)
    W_K = np.asarray(W_K, dtype=np.float32)
    W_V = np.asarray(W_V, dtype=np.float32)
    W_O = np.asarray(W_O, dtype=np.float32)
    b_Q = np.asarray(b_Q, dtype=np.float32)
    b_K = np.asarray(b_K, dtype=np.float32)
    b_V = np.asarray(b_V, dtype=np.float32)

    xT = [np.ascontiguousarray(x[b].T).astype(np_bf) for b in range(B)]
    # additive causal masks at k-group granularity: variant o covers the two
    # k-tiles at q-block offsets (2o*128, (2o+1)*128)
    kp = np.arange(P)[:, None]
    qc = np.arange(QB)[None, :]
    maskm = np.stack([
        np.concatenate([
            np.where(qc < (2 * o + j) * P + kp, np.float32(MASKV),
                     np.float32(0.0))
            for j in range(2)
        ], axis=1)
        for o in range(2)
    ]).astype(np_bf)
    idm = np.eye(P, dtype=np.float32).astype(np_bf)

    in_maps = []
    for c in range(NCORES):
        b = c // (NCORES // B)
        heads = [HPC * (c % (NCORES // B)) + i for i in range(HPC)]
        wq = np.concatenate([W_Q[h] for h in heads], axis=1).astype(np_bf)
        wk = np.concatenate([W_K[h] for h in heads], axis=1).astype(np_bf)
        # per head slot: [V_h | 0col]; the ones column comes from the bias
        zc = np.zeros((DM, 1), dtype=np.float32)
        wv = np.concatenate(
            sum(([W_V[h], zc] for h in heads), []), axis=1).astype(np_bf)
        wo = np.concatenate([W_O[h] for h in heads], axis=0).astype(np_bf)
        bq = np.stack([
            np.concatenate([b_Q[heads[0]], b_Q[heads[1]]]),
            np.concatenate([b_Q[heads[2]], b_Q[heads[3]]]),
        ]).astype(np.float32)
        bk = np.stack([
            np.concatenate([b_K[heads[0]], b_K[heads[1]]]),
            np.concatenate([b_K[heads[2]], b_K[heads[3]]]),
        ]).astype(np.float32)
        one = np.ones((1,), dtype=np.float32)
        bv = np.tile(np.concatenate(
            sum(([b_V[h], one] for h in heads), []))[None, :],
            (P, 1)).astype(np.float32)
        in_maps.append({
            "xT": np.ascontiguousarray(xT[b]),
            "wq": wq, "wk": wk, "wv": wv, "wo": wo,
            "bq": bq, "bk": bk, "bv": bv,
            "maskm": maskm, "idm": idm,
        })
    return in_maps


def kernel(normalized_resid_pre, W_Q, W_K, W_V, W_O, b_Q, b_K, b_V, b_O):
    global LAST_RESULTS
    nc = build_program()
    in_maps = make_in_maps(
        normalized_resid_pre, W_Q, W_K, W_V, W_O, b_Q, b_K, b_V, b_O
    )
    trace = os.environ.get("ATTN_TRACE", "0") == "1"
    res = run_bass_kernel_spmd(nc, in_maps, list(range(NCORES)), trace=trace)
    LAST_RESULTS = res

    b_O = np.asarray(b_O, dtype=np.float32)
    parts = [np.asarray(res.results[c]["out"], dtype=np.float64)
             for c in range(NCORES)]
    npc = NCORES // B  # cores per batch
    out = np.stack(
        [sum(parts[b * npc : (b + 1) * npc]) + b_O for b in range(B)]
    )
    return out.astype(np.float32)


# revision 43
# speedup vs baseline: 1.2420x; 1.1780x over previous
"""Trainium2 Bass kernel for causal multi-head attention (dense transformer).

Problem (hardcoded): x [2, 2048, 1024], 16 heads x 64 dh, causal, fp32 I/O.
Sharding: 8 cores = 2 batches x 4 head-groups. Each core computes 4 heads for
one batch plus a partial output projection [2048, 1024] (bf16); the host sums
the 4 partials per batch and adds b_O.

On-device dataflow (all transposed, no transposes anywhere):
  x^T (host-pretransposed)  ->  Q^T, K^T [dh, s] and V [s, dh] via matmuls
  S^T[k, q] = K Q^T  (+ causal -1e5 mask added via identity-matmul accum)
  P^T = exp(S^T / 8) on ACT straight into SBUF bf16 (no separate masking)
  Zu^T[dh, q] and the softmax denominator come from ONE matmul per k-tile:
    V is stored with a ones column appended ([V_A | 1] / [1 | V_B], 65 wide),
    so the PV matmul emits 64 rows of Z plus 1 row of sum(P).
  denominators: copied out, reciprocal'd, broadcast across partitions with a
    tiny ones-matmul, then Z is normalized while copying to SBUF.
  O[s, :] = (Z^T)^T W_O  (Z^T is directly the lhsT of the O-projection)

Heads are processed in pairs: QK^T packs 2 heads in row-groups (0-63/64-127)
of the PE array (concurrent); PV runs per head with 65-wide stationary weights.
"""

import os
from contextlib import ExitStack

import numpy as np

import concourse.tile as tile
from concourse import bacc, mybir
from concourse.bass_utils import run_bass_kernel_spmd

# problem constants
B, S, DM, H, DH = 2, 2048, 1024, 16, 64
P = 128          # partitions
QB = 512         # q block (matmul moving free dim)
NKT = S // P     # 16 k tiles
NQB = S // QB    # 4 q blocks
NDM = DM // P    # 8 d_model tiles
HPC = 4          # heads per core
NCORES = 8
MASKV = -99840.0  # additive causal mask (bf16-exact); exp(MASKV/8) == 0
VW = 2 * (DH + 1)  # 130: per-pair padded V width [V_A|0 , 0|V_B]

F32 = mybir.dt.float32
BF16 = mybir.dt.bfloat16
FP8 = mybir.dt.float8e4

_PROGRAM_CACHE = {}
LAST_RESULTS = None  # BassKernelResults of the most recent run (for test.py)

# packing bisect flags (hardware-debug)
PACK_UNITS = os.environ.get("ATTN_PACK_UNITS", "1") == "1"
PACK_SC = os.environ.get("ATTN_PACK_SC", "1") == "1"
PACK_MASK = os.environ.get("ATTN_PACK_MASK", "1") == "1"


def _mm(nc, out, lhsT, rhs, start, stop, skip=False, perf_mode=None):
    return nc.tensor.matmul(
        out, lhsT, rhs, start=start, stop=stop, skip_group_check=skip,
        perf_mode=perf_mode,
    )


def _chain(insts):
    """Ordering-only PE edges so row-group-packed matmuls stay adjacent and
    run concurrently on the array."""
    from concourse.tile import add_dep_helper

    for a, b in zip(insts[1:], insts):
        add_dep_helper(a.ins, b.ins, sync=False, reason="pack-pair order")


def build_program():
    key = (PACK_UNITS, PACK_SC, PACK_MASK)
    if key in _PROGRAM_CACHE:
        return _PROGRAM_CACHE[key]

    nc = bacc.Bacc(
        "TRN2", target_bir_lowering=False, debug=False, num_devices=NCORES
    )

    # ---- DRAM I/O (per-core shards, prearranged on host) ----
    xT_d = nc.dram_tensor("xT", [DM, S], BF16, kind="ExternalInput")
    wq_d = nc.dram_tensor("wq", [DM, HPC * DH], BF16, kind="ExternalInput")
    wk_d = nc.dram_tensor("wk", [DM, HPC * DH], BF16, kind="ExternalInput")
    wv_d = nc.dram_tensor("wv", [DM, 2 * VW], BF16, kind="ExternalInput")
    wo_d = nc.dram_tensor("wo", [HPC * DH, DM], BF16, kind="ExternalInput")
    bq_d = nc.dram_tensor("bq", [2, P], F32, kind="ExternalInput")
    bk_d = nc.dram_tensor("bk", [2, P], F32, kind="ExternalInput")
    bv_d = nc.dram_tensor("bv", [P, 2 * VW], F32, kind="ExternalInput")
    maskm_d = nc.dram_tensor("maskm", [2, P, 2 * QB], BF16, kind="ExternalInput")
    id_d = nc.dram_tensor("idm", [P, P], BF16, kind="ExternalInput")
    out_d = nc.dram_tensor("out", [S, DM], BF16, kind="ExternalOutput")

    with tile.TileContext(nc) as tc, ExitStack() as ctx:
        const = ctx.enter_context(tc.tile_pool(name="const", bufs=1))
        persist = ctx.enter_context(tc.tile_pool(name="persist", bufs=1))

        # ---- constants ----
        # row of ones at partition 64 (matmul lhsT/rhs must share base)
        ones1 = const.tile([DH + 1, DH], BF16, name="ones1", tag="ones1")
        nc.gpsimd.memset(ones1[DH : DH + 1, :], 1.0)
        maskm_sb = const.tile([P, 2, 2 * QB], BF16, name="maskm_sb", tag="maskm")
        id_sb = const.tile([P, P], BF16, name="id_sb", tag="idm")
        bq_sb = const.tile([P, 2], F32, name="bq_sb", tag="bq")
        bk_sb = const.tile([P, 2], F32, name="bk_sb", tag="bk")
        bv_sb = const.tile([P, 2 * VW], F32, name="bv_sb", tag="bv")

        # ---- persistent activations ----
        qt_sb = [persist.tile([P, S], BF16, name=f"qt{p}", tag=f"qt{p}")
                 for p in range(2)]
        kt_sb = [persist.tile([P, S], BF16, name=f"kt{p}", tag=f"kt{p}")
                 for p in range(2)]
        # V with ones column per head slot s: [:, kt, s, 0:64]=V, [:, kt, s, 64]=1
        v65_sb = [persist.tile([P, NKT, 2, DH + 1], BF16, name=f"v{p}",
                               tag=f"v{p}") for p in range(2)]
        zt_sb = [persist.tile([P, S], BF16, name=f"zt{p}", tag=f"zt{p}")
                 for p in range(2)]
        wo_sb = persist.tile([P, 2, DM], BF16, name="wo_sb", tag="wo")

        # ---- psum pools: sp (scores/proj/den_bc/O) 2x2 banks, zup 2x2 ----
        sp = ctx.enter_context(tc.tile_pool(name="sp", bufs=2, space="PSUM"))
        zup = ctx.enter_context(tc.tile_pool(name="zup", bufs=2, space="PSUM"))
        xw = ctx.enter_context(tc.tile_pool(name="xw", bufs=1))
        ppool = ctx.enter_context(tc.tile_pool(name="ppool", bufs=6))
        denp = ctx.enter_context(tc.tile_pool(name="denp", bufs=2))
        bcpool = ctx.enter_context(tc.tile_pool(name="bcpool", bufs=2))
        ost = ctx.enter_context(tc.tile_pool(name="ost", bufs=3))

        # ---- input DMAs: one big strided DMA per tensor / x column-chunk
        # (each dma_start costs ~0.65us of serial sync-engine issue time) ----
        xt_sb = xw.tile([P, NDM, S], BF16, name="xt_sb", tag="xt")
        w_sb = {
            wname: xw.tile([P, NDM, w], BF16, name=f"{wname}_sb", tag=wname)
            for wname, w in (("wq", HPC * DH), ("wk", HPC * DH), ("wv", 2 * VW))
        }

        def dma_w(wname, src):
            nc.sync.dma_start(
                out=w_sb[wname][:],
                in_=src[:, :].rearrange("(t p) c -> p t c", p=P))

        def dma_x(ch):
            nc.sync.dma_start(
                out=xt_sb[:, :, ch * QB : (ch + 1) * QB],
                in_=xT_d[:, ch * QB : (ch + 1) * QB].rearrange(
                    "(t p) s -> p t s", p=P))

        dma_w("wq", wq_d)
        dma_x(0)
        dma_w("wk", wk_d)
        nc.sync.dma_start(out=id_sb[:], in_=id_d[:, :])
        nc.sync.dma_start(out=maskm_sb[:],
                          in_=maskm_d[:, :, :].rearrange("o p c -> p o c"))
        nc.sync.dma_start(out=bq_sb[:], in_=bq_d[:, :].rearrange("a p -> p a"))
        nc.sync.dma_start(out=bk_sb[:], in_=bk_d[:, :].rearrange("a p -> p a"))
        nc.sync.dma_start(out=bv_sb[:], in_=bv_d[:, :])
        dma_w("wv", wv_d)
        for ch in range(1, NQB):
            dma_x(ch)
        nc.sync.dma_start(out=wo_sb[:],
                          in_=wo_d[:, :].rearrange("(pp p) d -> p pp d", p=P))

        def qk_unit(p, wname, dst, bias, ch):
            # Q^T or K^T chunk for pair p: [dh-pair (128), 512 seq cols];
            # rows 0-63 = head 2p, 64-127 = head 2p+1. Column-packed: the two
            # 64-wide output halves run concurrently on disjoint PE col-tiles.
            def emit():
                qp = sp.tile([P, 2, QB], F32, name="qp", tag="s")
                for t in range(NDM):
                    if PACK_UNITS:
                        _chain([
                            _mm(nc, qp[h * DH : (h + 1) * DH, 0, :],
                                w_sb[wname][:, t, p * P + h * DH : p * P + (h + 1) * DH],
                                xt_sb[:, t, ch * QB : (ch + 1) * QB],
                                start=(t == 0), stop=(t == NDM - 1), skip=True)
                            for h in range(2)
                        ])
                    else:
                        _mm(nc, qp[:, 0, :],
                            w_sb[wname][:, t, p * P : (p + 1) * P],
                            xt_sb[:, t, ch * QB : (ch + 1) * QB],
                            start=(t == 0), stop=(t == NDM - 1))
                nc.vector.tensor_scalar_add(
                    dst[p][:, ch * QB : (ch + 1) * QB], qp[:, 0, :],
                    bias[:, p : p + 1])
            return emit

        def v_unit(st):
            # V seq-tile st: [seq 128, per-head [V_h|0]]; the ones columns
            # come from the padded bias (wv cols are 0 there). Column-packed
            # over the two 64-row seq output halves.
            def emit():
                vp = sp.tile([P, 2, QB], F32, name="vp", tag="s")
                for t in range(NDM):
                    if PACK_UNITS:
                        _chain([
                            _mm(nc, vp[h * DH : (h + 1) * DH, 0, 0 : 2 * VW],
                                xt_sb[:, t, st * P + h * DH : st * P + (h + 1) * DH],
                                w_sb["wv"][:, t, :],
                                start=(t == 0), stop=(t == NDM - 1), skip=True)
                            for h in range(2)
                        ])
                    else:
                        _mm(nc, vp[:, 0, 0 : 2 * VW],
                            xt_sb[:, t, st * P : (st + 1) * P],
                            w_sb["wv"][:, t, :],
                            start=(t == 0), stop=(t == NDM - 1))
                for p in range(2):
                    nc.vector.tensor_add(
                        v65_sb[p][:, st, :, :],
                        vp[:, 0, p * VW : (p + 1) * VW].rearrange(
                            "p (a b) -> p a b", b=DH + 1),
                        bv_sb[:, p * VW : (p + 1) * VW].rearrange(
                            "p (a b) -> p a b", b=DH + 1))
            return emit

        def o_unit(st):
            # output-projection seq-tile st (both 512-col halves), column-
            # packed over the two 64-row seq output halves
            def emit():
                op = sp.tile([P, 2, QB], F32, name="op", tag="s")
                for nn in range(2):
                    for pp in range(2):
                        if PACK_UNITS:
                            _chain([
                                _mm(nc, op[h * DH : (h + 1) * DH, nn, :],
                                    zt_sb[pp][:, st * P + h * DH : st * P + (h + 1) * DH],
                                    wo_sb[:, pp, nn * QB : (nn + 1) * QB],
                                    start=(pp == 0), stop=(pp == 1), skip=True)
                                for h in range(2)
                            ])
                        else:
                            _mm(nc, op[:, nn, :],
                                zt_sb[pp][:, st * P : (st + 1) * P],
                                wo_sb[:, pp, nn * QB : (nn + 1) * QB],
                                start=(pp == 0), stop=(pp == 1))
                ot = ost.tile([P, 2, QB], BF16, name="ot", tag="ot")
                nc.vector.tensor_copy(ot[:], op[:])
                nc.gpsimd.dma_start(
                    out=out_d[st * P : (st + 1) * P, :],
                    in_=ot[:].rearrange("p a b -> p (a b)"))
            return emit

        def attn_qblock(p, qb, zu, fill, pre=None):
            """scores+mask -> exp -> PV for pair p, q-block qb, with 1-kg
            software pipelining (PV lags one k-group). `fill()` is called
            once per k-group to emit interleaved PE filler work; `pre` is
            emitted between the first sc/exp and the first PV (den chain of
            the other pair)."""
            q0 = qb * QB
            nk = (qb + 1) * (QB // P)     # k tiles in causal range
            prev = None                    # (pA, pB, kg) pending PV

            def pv(pA, pB, kg, c0kg):
                for j in range(2):
                    kt = kg * 2 + j
                    c0 = max(kt * P - q0, 0)
                    for s, px in ((0, pA), (1, pB)):
                        _mm(nc, zu[0 : DH + 1, s, c0:QB],
                            v65_sb[p][:, kt, s, :], px[:, j, c0:QB],
                            start=(kt == 0), stop=(kt == nk - 1))

            for kg in range(nk // 2):
                off0 = kg * 2 * P - q0    # first valid col of k-tile j=0
                band = off0 >= 0
                c0 = max(off0, 0)
                o = off0 // (2 * P) if band else 0
                sA = sp.tile([P, 2, QB], F32, name="sA", tag="s")
                sB = sp.tile([P, 2, QB], F32, name="sB", tag="s")
                for j in range(2):
                    if PACK_SC:
                        # quadrant-packed: (head-rows 0:64 / 64:128) x (out
                        # k-rows 0:64 / 64:128) = 4 concurrent tile positions
                        _chain([
                            _mm(nc, stile[h * DH : (h + 1) * DH, j, c0:QB],
                                kt_sb[p][rows,
                                         (kg * 2 + j) * P + h * DH :
                                         (kg * 2 + j) * P + (h + 1) * DH],
                                qt_sb[p][rows, q0 + c0 : q0 + QB],
                                start=True, stop=not band, skip=True)
                            for rows, stile in ((slice(0, 64), sA), (slice(64, P), sB))
                            for h in range(2)
                        ])
                    else:
                        _chain([
                            _mm(nc, stile[:, j, c0:QB],
                                kt_sb[p][rows, (kg * 2 + j) * P : (kg * 2 + j + 1) * P],
                                qt_sb[p][rows, q0 + c0 : q0 + QB],
                                start=True, stop=not band)
                            for rows, stile in ((slice(0, 64), sA), (slice(64, P), sB))
                        ])
                if band:
                    # additive causal mask via identity-matmul accumulation
                    for j in range(2):
                        for stile in (sA, sB):
                            if PACK_MASK and PACK_SC:
                                # row-packed: k-rows 0-63 / 64-127 concurrent
                                _chain([
                                    _mm(nc, stile[h * DH : (h + 1) * DH, j, c0:QB],
                                        id_sb[h * DH : (h + 1) * DH,
                                              h * DH : (h + 1) * DH],
                                        maskm_sb[h * DH : (h + 1) * DH, o,
                                                 j * QB + c0 : (j + 1) * QB],
                                        start=False, stop=True, skip=True)
                                    for h in range(2)
                                ])
                            else:
                                _mm(nc, stile[:, j, c0:QB], id_sb[:],
                                    maskm_sb[:, o, j * QB + c0 : (j + 1) * QB],
                                    start=False, stop=True, skip=True)
                pA = ppool.tile([P, 2, QB], BF16, name="pA", tag="pt")
                pB = ppool.tile([P, 2, QB], BF16, name="pB", tag="pt")
                nc.scalar.activation(pA[:, :, c0:QB], sA[:, :, c0:QB],
                                     mybir.ActivationFunctionType.Exp,
                                     scale=0.125)
                nc.scalar.activation(pB[:, :, c0:QB], sB[:, :, c0:QB],
                                     mybir.ActivationFunctionType.Exp,
                                     scale=0.125)
                if pre is not None:
                    pre()
                    pre = None
                fill()
                if prev is not None:
                    pv(*prev)
                prev = (pA, pB, kg, c0)
            pv(*prev)

        def den_chain(p, qb, zu):
            """denominator rows -> bf16 -> reciprocal of PE-broadcast -> Z."""
            q0 = qb * QB
            den16 = denp.tile([DH + 1, 2, QB], BF16, name="den16", tag="den")
            # zu[64,0]=den_A, zu[64,1]=den_B; copy at matching partition 64
            nc.vector.tensor_copy(den16[DH : DH + 1, :, :],
                                  zu[DH : DH + 1, :, :])
            dbc = sp.tile([P, 2, QB], F32, name="dbc", tag="s")
            _mm(nc, dbc[0:DH, 0, :], ones1[DH : DH + 1, :],
                den16[DH : DH + 1, 0, :], start=True, stop=True, skip=True)
            _mm(nc, dbc[DH:P, 0, :], ones1[DH : DH + 1, :],
                den16[DH : DH + 1, 1, :], start=True, stop=True, skip=True)
            bcr = bcpool.tile([P, QB], F32, name="bcr", tag="bcr")
            nc.vector.reciprocal_approx_fast(out=bcr[:], in_=dbc[:, 0, :])
            nc.vector.tensor_mul(zt_sb[p][0:DH, q0 : q0 + QB],
                                 zu[0:DH, 0, :], bcr[0:DH, :])
            nc.vector.tensor_mul(zt_sb[p][DH:P, q0 : q0 + QB],
                                 zu[0:DH, 1, :], bcr[DH:P, :])

        class Filler:
            """Evenly drains a unit list across an attn q-block's k-groups."""
            def __init__(self, units, slots):
                self.units = list(units)
                self.slots = slots

            def __call__(self):
                n = -(-len(self.units) // self.slots) if self.slots > 0 else 0
                for u in self.units[:n]:
                    u()
                del self.units[:n]
                self.slots -= 1

        def qkp(p, ch):
            return [qk_unit(p, "wq", qt_sb, bq_sb, ch),
                    qk_unit(p, "wk", kt_sb, bk_sb, ch)]

        # prologue: just enough projection for attn(0, qb0) to start
        for u in qkp(0, 0) + [v_unit(0), v_unit(1)]:
            u()

        # fills for [attn(0, qb), attn(1, qb)] segments; O(st) units appear
        # one q-block after their zt is finalized
        fills = {
            (0, 0): [v_unit(2), v_unit(3)] + qkp(1, 0),
            (1, 0): qkp(0, 1),
            (0, 1): qkp(1, 1) + [v_unit(s) for s in range(4, 8)],
            (1, 1): qkp(0, 2) + [o_unit(0), o_unit(1)],
            (0, 2): qkp(1, 2) + [v_unit(s) for s in range(8, 12)]
                    + [o_unit(2), o_unit(3)],
            (1, 2): qkp(0, 3) + [o_unit(4), o_unit(5)],
            (0, 3): qkp(1, 3) + [v_unit(s) for s in range(12, 16)]
                    + [o_unit(6), o_unit(7)],
            (1, 3): [o_unit(s) for s in range(8, 12)],
        }

        for qb in range(NQB):
            nslots = (qb + 1) * 2
            zu0 = zup.tile([P, 2, QB], F32, name="zu", tag="z")
            attn_qblock(0, qb, zu0, Filler(fills[(0, qb)], nslots))
            zu1 = zup.tile([P, 2, QB], F32, name="zu", tag="z")
            attn_qblock(1, qb, zu1, Filler(fills[(1, qb)], nslots),
                        pre=lambda q=qb, z=zu0: den_chain(0, q, z))
            den_chain(1, qb, zu1)
        for st in range(12, 16):
            o_unit(st)()

    nc.compile()
    _PROGRAM_CACHE["p"] = nc
    return nc


def make_in_maps(normalized_resid_pre, W_Q, W_K, W_V, W_O, b_Q, b_K, b_V, b_O):
    """Shard + prearrange the full inputs into per-core input maps."""
    import ml_dtypes  # noqa: F401  (registers bfloat16 with numpy)

    np_bf = np.dtype("bfloat16")
    x = np.asarray(normalized_resid_pre, dtype=np.float32)
    W_Q = np.asarray(W_Q, dtype=np.float32)
    W_K = np.asarray(W_K, dtype=np.float32)
    W_V = np.asarray(W_V, dtype=np.float32)
    W_O = np.asarray(W_O, dtype=np.float32)
    b_Q = np.asarray(b_Q, dtype=np.float32)
    b_K = np.asarray(b_K, dtype=np.float32)
    b_V = np.asarray(b_V, dtype=np.float32)

    xT = [np.ascontiguousarray(x[b].T).astype(np_bf) for b in range(B)]
    # additive causal masks at k-group granularity: variant o covers the two
    # k-tiles at q-block offsets (2o*128, (2o+1)*128)
    kp = np.arange(P)[:, None]
    qc = np.arange(QB)[None, :]
    maskm = np.stack([
        np.concatenate([
            np.where(qc < (2 * o + j) * P + kp, np.float32(MASKV),
                     np.float32(0.0))
            for j in range(2)
        ], axis=1)
        for o in range(2)
    ]).astype(np_bf)
    idm = np.eye(P, dtype=np.float32).astype(np_bf)

    in_maps = []
    for c in range(NCORES):
        b = c // (NCORES // B)
        heads = [HPC * (c % (NCORES // B)) + i for i in range(HPC)]
        wq = np.concatenate([W_Q[h] for h in heads], axis=1).astype(np_bf)
        wk = np.concatenate([W_K[h] for h in heads], axis=1).astype(np_bf)
        # per head slot: [V_h | 0col]; the ones column comes from the bias
        zc = np.zeros((DM, 1), dtype=np.float32)
        wv = np.concatenate(
            sum(([W_V[h], zc] for h in heads), []), axis=1).astype(np_bf)
        wo = np.concatenate([W_O[h] for h in heads], axis=0).astype(np_bf)
        bq = np.stack([
            np.concatenate([b_Q[heads[0]], b_Q[heads[1]]]),
            np.concatenate([b_Q[heads[2]], b_Q[heads[3]]]),
        ]).astype(np.float32)
        bk = np.stack([
            np.concatenate([b_K[heads[0]], b_K[heads[1]]]),
            np.concatenate([b_K[heads[2]], b_K[heads[3]]]),
        ]).astype(np.float32)
        one = np.ones((1,), dtype=np.float32)
        bv = np.tile(np.concatenate(
            sum(([b_V[h], one] for h in heads), []))[None, :],
            (P, 1)).astype(np.float32)
        in_maps.append({
            "xT": np.ascontiguousarray(xT[b]),
            "wq": wq, "wk": wk, "wv": wv, "wo": wo,
            "bq": bq, "bk": bk, "bv": bv,
            "maskm": maskm, "idm": idm,
        })
    return in_maps


def kernel(normalized_resid_pre, W_Q, W_K, W_V, W_O, b_Q, b_K, b_V, b_O):
    global LAST_RESULTS
    nc = build_program()
    in_maps = make_in_maps(
        normalized_resid_pre, W_Q, W_K, W_V, W_O, b_Q, b_K, b_V, b_O
    )
    trace = os.environ.get("ATTN_TRACE", "0") == "1"
    res = run_bass_kernel_spmd(nc, in_maps, list(range(NCORES)), trace=trace)
    LAST_RESULTS = res

    b_O = np.asarray(b_O, dtype=np.float32)
    parts = [np.asarray(res.results[c]["out"], dtype=np.float64)
             for c in range(NCORES)]
    npc = NCORES // B  # cores per batch
    out = np.stack(
        [sum(parts[b * npc : (b + 1) * npc]) + b_O for b in range(B)]
    )
    return out.astype(np.float32)
